# revision 22
# baseline (speedup 1.0000x reference)
"""Causal self-attention (S=2048, D=1024, 16 heads x 64) on 8 Trainium2 cores.

Tensor-parallel sharding: 2 heads per core. Each core computes
  qkv_local = x @ Wqkv[:, local]      (local q/k/v columns, q pre-scaled 1/8)
  attn_h    = softmax(mask(q_h k_h^T)) v_h          for its 2 heads
  partial   = concat(attn) @ Wout[local_rows, :]    (128 rows of Wout)
and the host sums the 8 bf16 partials (+bias).

v2 changes vs the first working kernel (measured +100ns/MM fixed overhead
= un-hidden LDWEIGHTS/dispatch; PE tiling amortizes it to ~4ns/tile):
 - logits are 4-way PE-tiled: head0/head1 contract over array rows
   0-63/64-127 (K=64 each, no zero padding) while each head's two 64-key
   halves drain to disjoint PSUM partitions (col tiles). tile_position is
   auto-derived from the operand base partitions.
 - qkv projection and output projection are 2-way col-tiled (M=64
   halves write disjoint partitions of the same PSUM bank, so the
   accumulation needs no combine step).
 - q^T/k^T live stacked in one [128, 2, S] tile (h0 rows 0-63, h1 rows
   64-127) straight from the projection PSUM layout; no pad memsets.
 - causal masking is narrowed to the one [128, 2heads, 128] sub-block
   per diagonal key-block that actually straddles the diagonal (the
   dead 128-query tail of the last diagonal block is not computed).
 - rowsum reciprocal reads the PSUM accumulator directly on DVE
   (drops the ACT staging copy).
 - qkv for chunk si+1 is emitted interleaved into the (ACT-bound)
   attention group pipeline of chunk si, budgeted so each chunk's
   producers are fully emitted before chunk si+1's consumers.

On-chip data is bf16 (PSUM accumulation fp32). Logits are computed
transposed ([key, query]) so exp(logits) feeds probs@v directly as the
moving operand; v carries a ones-column per head so the same matmul
accumulates softmax row-sums. No max-subtraction (logits ~N(0,1)).
"""

import numpy as np

import concourse.bass as bass
import concourse.mybir as mybir
import concourse.tile as tile
from concourse import bacc
from concourse.bass_utils import run_bass_kernel_spmd

S = 2048
D = 1024
DH = 64
N_CORES = 8

P = 128
NB512 = S // 512  # 512-wide query chunks
NB128 = S // 128  # 128-wide chunks
KO = D // P  # contraction chunks for the projections

F32 = mybir.dt.float32

_compiled = {}


def _emit(nc, tc, mm_dt, xt, w, wout, maskt, ident, out):
    f32 = F32
    with (
        tc.tile_pool(name="const", bufs=1) as const,
        tc.tile_pool(name="epool", bufs=4) as epool,
        tc.tile_pool(name="opool", bufs=6) as opool,
        tc.tile_pool(name="rcpool", bufs=1) as rcpool,
        tc.tile_pool(name="pslog", bufs=2, space="PSUM") as pslog,
        tc.tile_pool(name="psacc", bufs=2, space="PSUM") as psacc,
        tc.tile_pool(name="psmm", bufs=2, space="PSUM") as psmm,
    ):
        sb_xT = const.tile([P, KO, S], mm_dt, name="sb_xT")
        sb_w = const.tile([P, KO, 384], mm_dt, name="sb_w")
        sb_wout = const.tile([P, D], mm_dt, name="sb_wout")
        sb_mask = const.tile([P, 4, 512], mm_dt, name="sb_mask")
        # stacked q^T/k^T: [:, 0, :] = qT, [:, 1, :] = kT; head0 on
        # partitions 0-63, head1 on 64-127 (the projection PSUM layout)
        sb_qkT = const.tile([P, 2, S], mm_dt, name="sb_qkT")
        # v natural blocks, padded so the probs@v stationary can be a full
        # 128 columns (HAM only counts full-array matmuls as PE activity;
        # M=65 pv matmuls would let the clock gate re-throttle): h0 slice =
        # cols 0:128, h1 slice = cols 65:193; out partitions 65-127 are
        # garbage and never read. cols 130:193 are zeroed.
        sb_v = const.tile([P, NB128, 200], mm_dt, name="sb_v")
        sb_vT = const.tile([P, S], mm_dt, name="sb_vT")
        sb_attnT = const.tile([P, S], mm_dt, name="sb_attnT")
        sb_ident = const.tile([P, P], mm_dt, name="sb_ident")
        sb_warm = const.tile([P, 512], mm_dt, name="sb_warm")

        # PE warm-up: ~3.4us of dependency-free matmuls on a zero tile keep
        # the HAM activity window busy while the first input DMAs stream, so
        # the clock gate is fully open (2.4 GHz) when real matmuls start.
        nc.vector.memset(sb_warm[:], 0.0)
        for rows in [512] * 6 + [128] * 16:
            wps = psmm.tile([P, 512], f32, name="ps_warm", tag="mm")
            nc.tensor.matmul(
                wps[:, :rows], sb_warm[:, 0:P], sb_warm[:, :rows],
                start=True, stop=True,
            )

        # loads: small weights on the GpSimd SWDGE queue; xT column blocks
        # si-major so the first q/k chunk only waits on the first ~1MB
        for o in range(KO):
            weng = nc.scalar if o % 2 == 0 else nc.gpsimd
            weng.dma_start(sb_w[:, o, :], w[o * P : (o + 1) * P, :])
        xt3 = xt.rearrange("(o p) s -> p o s", p=P)
        for si in range(NB512):
            sl = slice(si * 512, (si + 1) * 512)
            for oo in range(0, KO, 2):
                nc.sync.dma_start(
                    sb_xT[:, oo : oo + 2, sl], xt3[:, oo : oo + 2, sl]
                )
        nc.gpsimd.dma_start(sb_mask[:], maskt[:])
        nc.gpsimd.dma_start(sb_wout[:], wout[:])
        nc.gpsimd.dma_start(sb_ident[:], ident[:])
        nc.gpsimd.memset(sb_v[:, :, DH], 1.0)
        nc.gpsimd.memset(sb_v[:, :, 129], 1.0)
        nc.gpsimd.memset(sb_v[:, :, 130:193], 0.0)

        # full-array heartbeat matmul: keeps the HAM clock gate fed during
        # stretches of partial-array (col/row-tiled) matmuls, which do not
        # register as PE activity
        def heartbeat():
            wps = pslog.tile([P, 1024], f32, name="ps_hb", tag="log")
            nc.tensor.matmul(
                wps[:, :64], sb_warm[:, 0:P], sb_warm[:, :64],
                start=True, stop=True,
            )

        # q^T/k^T producer, 2-way col-tiled: for each (o, q|k) the two
        # M=64 column halves run concurrently on disjoint PE col groups,
        # draining to disjoint partitions of the same PSUM bank.
        def emit_qk(si):
            ps = {}
            for cc in (0, 1):
                ps[cc] = psmm.tile([P, 512], f32, name="ps_qk", tag="mm")
            sl = slice(si * 512, (si + 1) * 512)
            for o in range(KO):
                for cc in (0, 1):
                    if o == 0:
                        # full-width first matmul: one clean bank-wide
                        # has_written clear (a col-tiled start would wipe
                        # the sibling half's accumulation state)
                        nc.tensor.matmul(
                            ps[cc][:],
                            sb_w[:, o, cc * P : (cc + 1) * P],
                            sb_xT[:, o, sl],
                            start=True,
                            stop=False,
                            skip_group_check=True,
                        )
                        continue
                    for hh in (0, 1):
                        nc.tensor.matmul(
                            ps[cc][hh * 64 : hh * 64 + 64, :],
                            sb_w[:, o, cc * P + hh * 64 : cc * P + hh * 64 + 64],
                            sb_xT[:, o, sl],
                            start=False,
                            stop=(o == KO - 1),
                            skip_group_check=True,
                        )
                if si == 0 and o % 2 == 1:
                    heartbeat()
                yield
            for cc in (0, 1):
                nc.vector.tensor_copy(sb_qkT[:, cc, sl], ps[cc][:])
            yield

        # v^T producer (same col-tiled shape), then PE-mode transposes turn
        # each 128x128 block into v natural layout
        def emit_vT(si):
            psv = psmm.tile([P, 512], f32, name="ps_vT", tag="mm")
            sl = slice(si * 512, (si + 1) * 512)
            for o in range(KO):
                if o == 0:
                    nc.tensor.matmul(
                        psv[:],
                        sb_w[:, o, 256:384],
                        sb_xT[:, o, sl],
                        start=True,
                        stop=False,
                        skip_group_check=True,
                    )
                    yield
                    continue
                for hh in (0, 1):
                    nc.tensor.matmul(
                        psv[hh * 64 : hh * 64 + 64, :],
                        sb_w[:, o, 256 + hh * 64 : 256 + hh * 64 + 64],
                        sb_xT[:, o, sl],
                        start=False,
                        stop=(o == KO - 1),
                        skip_group_check=True,
                    )
                if si == 0 and o % 2 == 1:
                    heartbeat()
                yield
            nc.vector.tensor_copy(sb_vT[:, sl], psv[:])
            yield

        def emit_v(sc):
            pt = psmm.tile([P, P], mm_dt, name="ps_t", tag="mm")
            nc.tensor.transpose(
                pt[:], sb_vT[:, sc * P : (sc + 1) * P], sb_ident[:]
            )
            nc.vector.tensor_copy(sb_v[:, sc, 0:DH], pt[:, 0:DH])
            nc.vector.tensor_copy(sb_v[:, sc, DH + 1 : 129], pt[:, DH:P])
            yield

        # output projection for one 128-row query chunk, col-tiled M=64;
        # PSUM escape alternates DVE/ACT to balance the two engines
        def emit_proj(sc):
            for ec in range(D // 512):
                pp = psacc.tile([P, 512], f32, name="ps_p", tag="acc")
                nc.tensor.matmul(
                    pp[:],
                    sb_attnT[:, sc * P : (sc + 1) * P],
                    sb_wout[:, ec * 512 : (ec + 1) * 512],
                    start=True,
                    stop=True,
                )
                ot = opool.tile([P, 512], mm_dt, name="ot", tag="ot")
                if (sc * 2 + ec) % 2 == 1:
                    nc.scalar.copy(ot[:], pp[:])
                else:
                    nc.vector.tensor_copy(ot[:], pp[:])
                nc.sync.dma_start(
                    out[sc * P : (sc + 1) * P, ec * 512 : (ec + 1) * 512], ot[:]
                )
                yield

        def drain(q, n):
            for _ in range(n):
                while q:
                    try:
                        next(q[0])
                        break
                    except StopIteration:
                        q.pop(0)
                if not q:
                    return

        chunk_q = []  # next chunk's qkv: must fully emit within this ic
        proj_q = []  # previous chunk's projections: drained opportunistically

        # ---- prologue: chunk 0 qkv, eagerly ----
        for gen in [emit_qk(0), emit_vT(0)] + [emit_v(sc) for sc in range(4)]:
            for _ in gen:
                pass

        # ---- attention over query chunks ----
        for ic in range(NB512):
            if ic + 1 < NB512:
                chunk_q = [emit_qk(ic + 1), emit_vT(ic + 1)]
                chunk_q += [emit_v(sc) for sc in range(4 * ic + 4, 4 * ic + 8)]
                chunk_steps = 23
            else:
                chunk_steps = 0
            njc = 4 * (ic + 1)
            # groups of 2 key-blocks sharing one [P,1024] PSUM tile per
            # head; diagonal blocks narrowed to causal-live query columns
            groups = []  # list of [(jc, col_start, n, i0), ...]
            for jp in range(2 * ic):
                groups.append([(2 * jp, 0, 512, 0), (2 * jp + 1, 512, 512, 0)])
            groups.append([(4 * ic, 0, 512, 0), (4 * ic + 1, 512, 384, 128)])
            groups.append([(4 * ic + 2, 0, 256, 256), (4 * ic + 3, 256, 128, 384)])
            per_group = -(-chunk_steps // len(groups)) if chunk_steps else 0

            def emit_pv(entries, e):
                for jc, cs, n, i0 in entries:
                    for h in (0, 1):
                        nc.tensor.matmul(
                            acc[h][:, i0 : i0 + n],
                            sb_v[:, jc, h * 65 : h * 65 + 128],
                            e[:, h, cs : cs + n],
                            start=(jc == 0),
                            stop=(jc == njc - 1),
                            skip_group_check=True,
                        )

            acc = {}
            for h in (0, 1):
                acc[h] = psacc.tile([P, 512], f32, name="ps_acc", tag="acc")
            pend = None  # (entries, e_tile) awaiting probs@v emission
            for grp in groups:
                tot = grp[-1][1] + grp[-1][2]
                L = {}
                for h in (0, 1):
                    L[h] = pslog.tile([P, 1024], f32, name="ps_l", tag="log")
                # 4-way tiled logits: (array rows 64h..64h+63) x (col group
                # kk) -> PSUM partitions 64kk..64kk+63 of L[h]
                for jc, cs, n, i0 in grp:
                    for h in (0, 1):
                        for kk in (0, 1):
                            nc.tensor.matmul(
                                L[h][kk * 64 : kk * 64 + 64, cs : cs + n],
                                sb_qkT[
                                    h * 64 : h * 64 + 64,
                                    1,
                                    jc * P + kk * 64 : jc * P + kk * 64 + 64,
                                ],
                                sb_qkT[
                                    h * 64 : h * 64 + 64,
                                    0,
                                    ic * 512 + i0 : ic * 512 + i0 + n,
                                ],
                                start=True,
                                stop=True,
                                skip_group_check=True,
                            )
                e = epool.tile([P, 2, 1024], mm_dt, name="e_t", tag="e")
                for h in (0, 1):
                    nc.scalar.activation(
                        e[:, h, :tot], L[h][:, :tot],
                        mybir.ActivationFunctionType.Exp,
                    )
                # narrowed causal mask: only the [128,128] query sub-block
                # straddling each diagonal key-block needs masking
                for jc, cs, n, i0 in grp:
                    r = jc - 4 * ic
                    if r >= 0:
                        c0 = cs + (128 * r - i0)
                        m = sb_mask[:, r : r + 1, 128 * r : 128 * r + 128]
                        nc.vector.tensor_mul(
                            e[:, :, c0 : c0 + 128],
                            e[:, :, c0 : c0 + 128],
                            m.to_broadcast([P, 2, 128]),
                        )
                drain(chunk_q, per_group)
                if pend is not None:
                    emit_pv(*pend)
                pend = (grp, e)
                drain(proj_q, 2)
            emit_pv(*pend)
            e_last = pend[1]
            drain(chunk_q, 99)

            if ic == NB512 - 1:
                # bridge the normalize latency before the final projections
                # so the PE clock gate stays open for the tail. The moving
                # operand reads the last exp tile: a dependency-free spam
                # matmul would be hoisted early by the tile scheduler.
                for _ in range(12):
                    wps = psmm.tile([P, 512], f32, name="ps_warm", tag="mm")
                    nc.tensor.matmul(
                        wps[:, :256], sb_warm[:, 0:P], e_last[:, 0, :256],
                        start=True, stop=True,
                    )

            # normalize: reciprocal of the PSUM rowsum row, broadcast across
            # partitions on GpSimd, then one PSUM-reading multiply into attnT
            for h in (0, 1):
                rsk = rcpool.tile([1, 512], f32, name="rsk", tag="rsk", bufs=2)
                # reciprocal_approx_fast ignores a PSUM source's base
                # partition; stage the rowsum row to SBUF partition 0 first
                # (ACT for h1 at the tail so the two stages run in parallel)
                if ic == NB512 - 1 and h == 1:
                    nc.scalar.copy(rsk[:], acc[h][DH : DH + 1, :])
                else:
                    nc.vector.tensor_copy(rsk[:], acc[h][DH : DH + 1, :])
                rck = rcpool.tile([1, 512], f32, name="rck", tag="rck", bufs=3)
                nc.vector.reciprocal_approx_fast(rck[:], rsk[:])
                bck = rcpool.tile([DH, 512], f32, name="bck", tag="bck", bufs=3)
                nc.gpsimd.partition_broadcast(bck[:], rck[:])
                dst = sb_attnT[h * DH : h * DH + DH, ic * 512 : (ic + 1) * 512]
                nc.vector.tensor_mul(dst, acc[h][0:DH, :], bck[:])
            proj_q += [emit_proj(sc) for sc in range(4 * ic, 4 * (ic + 1))]
            drain(proj_q, 2)
        drain(proj_q, 999)


def build(mm_dt=mybir.dt.bfloat16):
    key = str(mm_dt)
    if key in _compiled:
        return _compiled[key]
    nc = bacc.Bacc("TRN2", target_bir_lowering=False, debug=False, num_devices=N_CORES)
    xt = nc.dram_tensor("xt", [D, S], mm_dt, kind="ExternalInput").ap()
    w = nc.dram_tensor("w", [D, 384], mm_dt, kind="ExternalInput").ap()
    wout = nc.dram_tensor("wout", [P, D], mm_dt, kind="ExternalInput").ap()
    maskt = nc.dram_tensor("maskt", [P, 4, 512], mm_dt, kind="ExternalInput").ap()
    ident = nc.dram_tensor("ident", [P, P], mm_dt, kind="ExternalInput").ap()
    out = nc.dram_tensor("out", [S, D], mm_dt, kind="ExternalOutput").ap()
    with tile.TileContext(nc) as tc:
        _emit(nc, tc, mm_dt, xt, w, wout, maskt, ident, out)
    nc.compile()
    _compiled[key] = nc
    return nc


def _np_dt(mm_dt):
    if mm_dt == mybir.dt.bfloat16:
        import ml_dtypes

        return ml_dtypes.bfloat16
    return np.float32


def make_inputs(x, Wqkv, Wout, np_dt):
    """Host-side shard/layout prep -> per-core input maps."""
    x = np.ascontiguousarray(np.asarray(x, np.float32))
    Wqkv = np.asarray(Wqkv, np.float32)
    Wout = np.asarray(Wout, np.float32)
    xT = np.ascontiguousarray(x.T).astype(np_dt)  # [D, S]
    j = np.arange(512, dtype=np.int64)
    m512 = (j[:, None] <= j[None, :]).astype(np.float32)  # [J, i]: J <= i
    mask = np.ascontiguousarray(
        m512.reshape(4, 128, 512).transpose(1, 0, 2)
    ).astype(np_dt)  # [p, r, i] = (128r + p <= i)
    in_maps = []
    for c in range(N_CORES):
        wq = Wqkv[:, 128 * c : 128 * (c + 1)] * (1.0 / np.sqrt(DH))
        wk = Wqkv[:, D + 128 * c : D + 128 * (c + 1)]
        wv = Wqkv[:, 2 * D + 128 * c : 2 * D + 128 * (c + 1)]
        w_loc = np.ascontiguousarray(np.concatenate([wq, wk, wv], axis=1))
        wout_loc = np.ascontiguousarray(Wout[128 * c : 128 * (c + 1), :])
        in_maps.append(
            {
                "xt": xT,
                "w": w_loc.astype(np_dt),
                "wout": wout_loc.astype(np_dt),
                "maskt": mask,
                "ident": np.eye(P, dtype=np_dt),
            }
        )
    return in_maps


def kernel(x, Wqkv, Wout, bias, mm_dt=mybir.dt.bfloat16, **run_kwargs):
    nc = build(mm_dt)
    in_maps = make_inputs(x, Wqkv, Wout, _np_dt(mm_dt))
    res = run_bass_kernel_spmd(nc, in_maps, core_ids=list(range(N_CORES)), **run_kwargs)
    acc = np.zeros((S, D), np.float64)
    for c in range(N_CORES):
        acc += res.results[c]["out"].astype(np.float64)
    acc += np.asarray(bias, np.float64)[None, :]
    return acc.astype(np.float32)


# revision 24
# speedup vs baseline: 1.1902x; 1.1902x over previous
"""Causal self-attention (S=2048, D=1024, 16 heads x 64) on 8 Trainium2 cores.

Tensor-parallel sharding: 2 heads per core. Each core computes
  qkv_local = x @ Wqkv[:, local]      (local q/k/v columns, q pre-scaled 1/8)
  attn_h    = softmax(mask(q_h k_h^T)) v_h          for its 2 heads
  partial   = concat(attn) @ Wout[local_rows, :]    (128 rows of Wout)
and the host sums the 8 bf16 partials (+bias).

v2 changes vs the first working kernel (measured +100ns/MM fixed overhead
= un-hidden LDWEIGHTS/dispatch; PE tiling amortizes it to ~4ns/tile):
 - logits are 4-way PE-tiled: head0/head1 contract over array rows
   0-63/64-127 (K=64 each, no zero padding) while each head's two 64-key
   halves drain to disjoint PSUM partitions (col tiles). tile_position is
   auto-derived from the operand base partitions.
 - qkv projection and output projection are 2-way col-tiled (M=64
   halves write disjoint partitions of the same PSUM bank, so the
   accumulation needs no combine step).
 - q^T/k^T live stacked in one [128, 2, S] tile (h0 rows 0-63, h1 rows
   64-127) straight from the projection PSUM layout; no pad memsets.
 - causal masking is narrowed to the one [128, 2heads, 128] sub-block
   per diagonal key-block that actually straddles the diagonal (the
   dead 128-query tail of the last diagonal block is not computed).
 - rowsum reciprocal reads the PSUM accumulator directly on DVE
   (drops the ACT staging copy).
 - qkv for chunk si+1 is emitted interleaved into the (ACT-bound)
   attention group pipeline of chunk si, budgeted so each chunk's
   producers are fully emitted before chunk si+1's consumers.

On-chip data is bf16 (PSUM accumulation fp32). Logits are computed
transposed ([key, query]) so exp(logits) feeds probs@v directly as the
moving operand; v carries a ones-column per head so the same matmul
accumulates softmax row-sums. No max-subtraction (logits ~N(0,1)).
"""

import numpy as np

import concourse.bass as bass
import concourse.mybir as mybir
import concourse.tile as tile
from concourse import bacc
from concourse.bass_utils import run_bass_kernel_spmd

S = 2048
D = 1024
DH = 64
N_CORES = 8

P = 128
NB512 = S // 512  # 512-wide query chunks
NB128 = S // 128  # 128-wide chunks
KO = D // P  # contraction chunks for the projections

F32 = mybir.dt.float32

_compiled = {}


def _emit(nc, tc, mm_dt, xt, w, wout, maskt, ident, out):
    f32 = F32
    with (
        tc.tile_pool(name="const", bufs=1) as const,
        tc.tile_pool(name="epool", bufs=4) as epool,
        tc.tile_pool(name="opool", bufs=6) as opool,
        tc.tile_pool(name="rcpool", bufs=1) as rcpool,
        tc.tile_pool(name="pslog", bufs=2, space="PSUM") as pslog,
        tc.tile_pool(name="psacc", bufs=2, space="PSUM") as psacc,
        tc.tile_pool(name="psmm", bufs=2, space="PSUM") as psmm,
    ):
        sb_xT = const.tile([P, KO, S], mm_dt, name="sb_xT")
        sb_w = const.tile([P, KO, 384], mm_dt, name="sb_w")
        sb_wout = const.tile([P, D], mm_dt, name="sb_wout")
        sb_mask = const.tile([P, 4, 512], mm_dt, name="sb_mask")
        # stacked q^T/k^T: [:, 0, :] = qT, [:, 1, :] = kT; head0 on
        # partitions 0-63, head1 on 64-127 (the projection PSUM layout)
        sb_qkT = const.tile([P, 2, S], mm_dt, name="sb_qkT")
        # v natural blocks, padded so the probs@v stationary can be a full
        # 128 columns (HAM only counts full-array matmuls as PE activity;
        # M=65 pv matmuls would let the clock gate re-throttle): h0 slice =
        # cols 0:128, h1 slice = cols 65:193; out partitions 65-127 are
        # garbage and never read. cols 130:193 are zeroed.
        sb_v = const.tile([P, NB128, 200], mm_dt, name="sb_v")
        sb_vT = const.tile([P, S], mm_dt, name="sb_vT")
        sb_attnT = const.tile([P, S], mm_dt, name="sb_attnT")
        sb_ident = const.tile([P, P], mm_dt, name="sb_ident")
        sb_warm = const.tile([P, 512], mm_dt, name="sb_warm")

        # PE warm-up: ~3.4us of dependency-free matmuls on a zero tile keep
        # the HAM activity window busy while the first input DMAs stream, so
        # the clock gate is fully open (2.4 GHz) when real matmuls start.
        nc.vector.memset(sb_warm[:], 0.0)
        for rows in [512] * 6 + [128] * 16:
            wps = psmm.tile([P, 512], f32, name="ps_warm", tag="mm")
            nc.tensor.matmul(
                wps[:, :rows], sb_warm[:, 0:P], sb_warm[:, :rows],
                start=True, stop=True,
            )

        # loads: small weights on the GpSimd SWDGE queue; xT column blocks
        # si-major so the first q/k chunk only waits on the first ~1MB
        for o in range(KO):
            weng = nc.scalar if o % 2 == 0 else nc.gpsimd
            weng.dma_start(sb_w[:, o, :], w[o * P : (o + 1) * P, :])
        xt3 = xt.rearrange("(o p) s -> p o s", p=P)
        for si in range(NB512):
            sl = slice(si * 512, (si + 1) * 512)
            for oo in range(0, KO, 2):
                nc.sync.dma_start(
                    sb_xT[:, oo : oo + 2, sl], xt3[:, oo : oo + 2, sl]
                )
        nc.gpsimd.dma_start(sb_mask[:], maskt[:])
        nc.gpsimd.dma_start(sb_wout[:], wout[:])
        nc.gpsimd.dma_start(sb_ident[:], ident[:])
        nc.gpsimd.memset(sb_v[:, :, DH], 1.0)
        nc.gpsimd.memset(sb_v[:, :, 129], 1.0)
        nc.gpsimd.memset(sb_v[:, :, 130:193], 0.0)

        # full-array heartbeat matmul: keeps the HAM clock gate fed during
        # stretches of partial-array (col/row-tiled) matmuls, which do not
        # register as PE activity
        def heartbeat():
            wps = pslog.tile([P, 1024], f32, name="ps_hb", tag="log")
            nc.tensor.matmul(
                wps[:, :64], sb_warm[:, 0:P], sb_warm[:, :64],
                start=True, stop=True,
            )

        # q^T/k^T producer, 2-way col-tiled: for each (o, q|k) the two
        # M=64 column halves run concurrently on disjoint PE col groups,
        # draining to disjoint partitions of the same PSUM bank.
        def emit_qk(si):
            ps = {}
            for cc in (0, 1):
                ps[cc] = psmm.tile([P, 512], f32, name="ps_qk", tag="mm")
            sl = slice(si * 512, (si + 1) * 512)
            for o in range(KO):
                for cc in (0, 1):
                    if o == 0:
                        # full-width first matmul: one clean bank-wide
                        # has_written clear (a col-tiled start would wipe
                        # the sibling half's accumulation state)
                        nc.tensor.matmul(
                            ps[cc][:],
                            sb_w[:, o, cc * P : (cc + 1) * P],
                            sb_xT[:, o, sl],
                            start=True,
                            stop=False,
                            skip_group_check=True,
                        )
                        continue
                    for hh in (0, 1):
                        nc.tensor.matmul(
                            ps[cc][hh * 64 : hh * 64 + 64, :],
                            sb_w[:, o, cc * P + hh * 64 : cc * P + hh * 64 + 64],
                            sb_xT[:, o, sl],
                            start=False,
                            stop=(o == KO - 1),
                            skip_group_check=True,
                        )
                if si == 0 and o % 2 == 1:
                    heartbeat()
                yield
            for cc in (0, 1):
                nc.vector.tensor_copy(sb_qkT[:, cc, sl], ps[cc][:])
            yield

        # v^T producer (same col-tiled shape), then PE-mode transposes turn
        # each 128x128 block into v natural layout
        def emit_vT(si):
            psv = psmm.tile([P, 512], f32, name="ps_vT", tag="mm")
            sl = slice(si * 512, (si + 1) * 512)
            for o in range(KO):
                if o == 0:
                    nc.tensor.matmul(
                        psv[:],
                        sb_w[:, o, 256:384],
                        sb_xT[:, o, sl],
                        start=True,
                        stop=False,
                        skip_group_check=True,
                    )
                    yield
                    continue
                for hh in (0, 1):
                    nc.tensor.matmul(
                        psv[hh * 64 : hh * 64 + 64, :],
                        sb_w[:, o, 256 + hh * 64 : 256 + hh * 64 + 64],
                        sb_xT[:, o, sl],
                        start=False,
                        stop=(o == KO - 1),
                        skip_group_check=True,
                    )
                if si == 0 and o % 2 == 1:
                    heartbeat()
                yield
            nc.vector.tensor_copy(sb_vT[:, sl], psv[:])
            yield

        def emit_v(sc):
            pt = psmm.tile([P, P], mm_dt, name="ps_t", tag="mm")
            nc.tensor.transpose(
                pt[:], sb_vT[:, sc * P : (sc + 1) * P], sb_ident[:]
            )
            nc.vector.tensor_copy(sb_v[:, sc, 0:DH], pt[:, 0:DH])
            nc.vector.tensor_copy(sb_v[:, sc, DH + 1 : 129], pt[:, DH:P])
            yield

        # output projection for one 128-row query chunk, col-tiled M=64;
        # PSUM escape alternates DVE/ACT to balance the two engines
        def emit_proj(sc):
            for ec in range(D // 512):
                pp = psacc.tile([P, 512], f32, name="ps_p", tag="acc")
                nc.tensor.matmul(
                    pp[:],
                    sb_attnT[:, sc * P : (sc + 1) * P],
                    sb_wout[:, ec * 512 : (ec + 1) * 512],
                    start=True,
                    stop=True,
                )
                ot = opool.tile([P, 512], mm_dt, name="ot", tag="ot")
                if (sc * 2 + ec) % 2 == 1:
                    nc.scalar.copy(ot[:], pp[:])
                else:
                    nc.vector.tensor_copy(ot[:], pp[:])
                nc.sync.dma_start(
                    out[sc * P : (sc + 1) * P, ec * 512 : (ec + 1) * 512], ot[:]
                )
                yield

        def drain(q, n):
            for _ in range(n):
                while q:
                    try:
                        next(q[0])
                        break
                    except StopIteration:
                        q.pop(0)
                if not q:
                    return

        chunk_q = []  # next chunk's qkv: must fully emit within this ic
        proj_q = []  # previous chunk's projections: drained opportunistically

        # ---- prologue: chunk 0 qkv, eagerly ----
        for gen in [emit_qk(0), emit_vT(0)] + [emit_v(sc) for sc in range(4)]:
            for _ in gen:
                pass

        # ---- attention over query chunks ----
        for ic in range(NB512):
            if ic + 1 < NB512:
                chunk_q = [emit_qk(ic + 1), emit_vT(ic + 1)]
                chunk_q += [emit_v(sc) for sc in range(4 * ic + 4, 4 * ic + 8)]
                chunk_steps = 23
            else:
                chunk_steps = 0
            njc = 4 * (ic + 1)
            # groups of 2 key-blocks sharing one [P,1024] PSUM tile per
            # head; diagonal blocks narrowed to causal-live query columns
            groups = []  # list of [(jc, col_start, n, i0), ...]
            for jp in range(2 * ic):
                groups.append([(2 * jp, 0, 512, 0), (2 * jp + 1, 512, 512, 0)])
            groups.append([(4 * ic, 0, 512, 0), (4 * ic + 1, 512, 384, 128)])
            groups.append([(4 * ic + 2, 0, 256, 256), (4 * ic + 3, 256, 128, 384)])
            per_group = -(-chunk_steps // len(groups)) if chunk_steps else 0

            def emit_pv(entries, e):
                for jc, cs, n, i0 in entries:
                    for h in (0, 1):
                        nc.tensor.matmul(
                            acc[h][:, i0 : i0 + n],
                            sb_v[:, jc, h * 65 : h * 65 + 128],
                            e[:, h, cs : cs + n],
                            start=(jc == 0),
                            stop=(jc == njc - 1),
                            skip_group_check=True,
                        )

            acc = {}
            for h in (0, 1):
                acc[h] = psacc.tile([P, 512], f32, name="ps_acc", tag="acc")
            pend = None  # (entries, e_tile) awaiting probs@v emission
            for grp in groups:
                tot = grp[-1][1] + grp[-1][2]
                L = {}
                for h in (0, 1):
                    L[h] = pslog.tile([P, 1024], f32, name="ps_l", tag="log")
                # 4-way tiled logits: (array rows 64h..64h+63) x (col group
                # kk) -> PSUM partitions 64kk..64kk+63 of L[h]
                for jc, cs, n, i0 in grp:
                    for h in (0, 1):
                        for kk in (0, 1):
                            nc.tensor.matmul(
                                L[h][kk * 64 : kk * 64 + 64, cs : cs + n],
                                sb_qkT[
                                    h * 64 : h * 64 + 64,
                                    1,
                                    jc * P + kk * 64 : jc * P + kk * 64 + 64,
                                ],
                                sb_qkT[
                                    h * 64 : h * 64 + 64,
                                    0,
                                    ic * 512 + i0 : ic * 512 + i0 + n,
                                ],
                                start=True,
                                stop=True,
                                skip_group_check=True,
                            )
                e = epool.tile([P, 2, 1024], mm_dt, name="e_t", tag="e")
                for h in (0, 1):
                    nc.scalar.activation(
                        e[:, h, :tot], L[h][:, :tot],
                        mybir.ActivationFunctionType.Exp,
                    )
                # narrowed causal mask: only the [128,128] query sub-block
                # straddling each diagonal key-block needs masking
                for jc, cs, n, i0 in grp:
                    r = jc - 4 * ic
                    if r >= 0:
                        c0 = cs + (128 * r - i0)
                        m = sb_mask[:, r : r + 1, 128 * r : 128 * r + 128]
                        nc.vector.tensor_mul(
                            e[:, :, c0 : c0 + 128],
                            e[:, :, c0 : c0 + 128],
                            m.to_broadcast([P, 2, 128]),
                        )
                drain(chunk_q, per_group)
                if pend is not None:
                    emit_pv(*pend)
                pend = (grp, e)
                drain(proj_q, 2)
            emit_pv(*pend)
            e_last = pend[1]
            drain(chunk_q, 99)

            # bridge the normalize latency at EVERY chunk boundary so the
            # PE clock gate stays open (the ic2->ic3 boundary measurably
            # re-throttled for 6.8us). The moving operand reads the last
            # exp tile: a dependency-free spam matmul would be hoisted
            # early by the tile scheduler.
            for _ in range(12 if ic == NB512 - 1 else 6):
                wps = psmm.tile([P, 512], f32, name="ps_warm", tag="mm")
                nc.tensor.matmul(
                    wps[:, :256], sb_warm[:, 0:P], e_last[:, 0, :256],
                    start=True, stop=True,
                )

            # normalize: reciprocal of the PSUM rowsum row, broadcast across
            # partitions on GpSimd, then one PSUM-reading multiply into attnT
            for h in (0, 1):
                rsk = rcpool.tile([1, 512], f32, name="rsk", tag="rsk", bufs=2)
                # reciprocal_approx_fast ignores a PSUM source's base
                # partition; stage the rowsum row to SBUF partition 0 first
                # (ACT for h1 so the two stages run in parallel)
                if h == 1:
                    nc.scalar.copy(rsk[:], acc[h][DH : DH + 1, :])
                else:
                    nc.vector.tensor_copy(rsk[:], acc[h][DH : DH + 1, :])
                rck = rcpool.tile([1, 512], f32, name="rck", tag="rck", bufs=3)
                nc.vector.reciprocal_approx_fast(rck[:], rsk[:])
                bck = rcpool.tile([DH, 512], f32, name="bck", tag="bck", bufs=3)
                nc.gpsimd.partition_broadcast(bck[:], rck[:])
                dst = sb_attnT[h * DH : h * DH + DH, ic * 512 : (ic + 1) * 512]
                nc.vector.tensor_mul(dst, acc[h][0:DH, :], bck[:])
            proj_q += [emit_proj(sc) for sc in range(4 * ic, 4 * (ic + 1))]
            drain(proj_q, 2)
        drain(proj_q, 999)
        # tail spam: ready only after the final normalize (reads attnT), so
        # it fills the escape-paced gaps of the last projections instead of
        # being hoisted; the final proj tail measurably ran at K=4/8 without
        # this (rethrottle at 98us for the whole 8us projection chain)
        for _ in range(12):
            wps = psmm.tile([P, 512], f32, name="ps_warm", tag="mm")
            nc.tensor.matmul(
                wps[:, :384], sb_warm[:, 0:P], sb_attnT[:, 1536:1920],
                start=True, stop=True,
            )


def build(mm_dt=mybir.dt.bfloat16):
    key = str(mm_dt)
    if key in _compiled:
        return _compiled[key]
    nc = bacc.Bacc("TRN2", target_bir_lowering=False, debug=False, num_devices=N_CORES)
    xt = nc.dram_tensor("xt", [D, S], mm_dt, kind="ExternalInput").ap()
    w = nc.dram_tensor("w", [D, 384], mm_dt, kind="ExternalInput").ap()
    wout = nc.dram_tensor("wout", [P, D], mm_dt, kind="ExternalInput").ap()
    maskt = nc.dram_tensor("maskt", [P, 4, 512], mm_dt, kind="ExternalInput").ap()
    ident = nc.dram_tensor("ident", [P, P], mm_dt, kind="ExternalInput").ap()
    out = nc.dram_tensor("out", [S, D], mm_dt, kind="ExternalOutput").ap()
    with tile.TileContext(nc) as tc:
        _emit(nc, tc, mm_dt, xt, w, wout, maskt, ident, out)
    nc.compile()
    _compiled[key] = nc
    return nc


def _np_dt(mm_dt):
    if mm_dt == mybir.dt.bfloat16:
        import ml_dtypes

        return ml_dtypes.bfloat16
    return np.float32


def make_inputs(x, Wqkv, Wout, np_dt):
    """Host-side shard/layout prep -> per-core input maps."""
    x = np.ascontiguousarray(np.asarray(x, np.float32))
    Wqkv = np.asarray(Wqkv, np.float32)
    Wout = np.asarray(Wout, np.float32)
    xT = np.ascontiguousarray(x.T).astype(np_dt)  # [D, S]
    j = np.arange(512, dtype=np.int64)
    m512 = (j[:, None] <= j[None, :]).astype(np.float32)  # [J, i]: J <= i
    mask = np.ascontiguousarray(
        m512.reshape(4, 128, 512).transpose(1, 0, 2)
    ).astype(np_dt)  # [p, r, i] = (128r + p <= i)
    in_maps = []
    for c in range(N_CORES):
        wq = Wqkv[:, 128 * c : 128 * (c + 1)] * (1.0 / np.sqrt(DH))
        wk = Wqkv[:, D + 128 * c : D + 128 * (c + 1)]
        wv = Wqkv[:, 2 * D + 128 * c : 2 * D + 128 * (c + 1)]
        w_loc = np.ascontiguousarray(np.concatenate([wq, wk, wv], axis=1))
        wout_loc = np.ascontiguousarray(Wout[128 * c : 128 * (c + 1), :])
        in_maps.append(
            {
                "xt": xT,
                "w": w_loc.astype(np_dt),
                "wout": wout_loc.astype(np_dt),
                "maskt": mask,
                "ident": np.eye(P, dtype=np_dt),
            }
        )
    return in_maps


def kernel(x, Wqkv, Wout, bias, mm_dt=mybir.dt.bfloat16, **run_kwargs):
    nc = build(mm_dt)
    in_maps = make_inputs(x, Wqkv, Wout, _np_dt(mm_dt))
    res = run_bass_kernel_spmd(nc, in_maps, core_ids=list(range(N_CORES)), **run_kwargs)
    acc = np.zeros((S, D), np.float64)
    for c in range(N_CORES):
        acc += res.results[c]["out"].astype(np.float64)
    acc += np.asarray(bias, np.float64)[None, :]
    return acc.astype(np.float32)
